# revision 1
# baseline (speedup 1.0000x reference)
"""Bahdanau attention Trainium2 kernel.

  h_exp   = (hidden @ W_h)[:, None, :]             # [B,1,H]
  f_proj  = features @ W_f                         # [B,L,H]
  energy  = einsum('blh,h->bl', tanh(h_exp+f_proj), V)
  weights = softmax(energy, axis=1)                # [B,L]
  context = einsum('bl,blf->bf', weights, features)

Sharding: data-parallel over batch B=32 across 8 NeuronCores (4 batches
per core); W_h/W_f/V replicated. Inputs are cast to bf16 on the host;
all matmuls run in bf16 with fp32 PSUM accumulation; softmax runs in
fp32 on-chip.

Per-core dataflow (R = 4*2048 = 8192 rows, rg = 512-row groups):
  - f_projT tile [128 H, 512 rows] = sum_k W_f[k,m].T @ featT[k,rg]
    (featT comes from an HBM DMA-transpose load of bf16 features)
  - ScalarE: t = tanh(psum + h_projT[:,m,b])  (bias trick, PSUM->SBUF)
  - PE: psum_e[1,512] += V[m].T @ t            (accumulate over m)
  - softmax per batch on [1,2048] (ACT Exp with accum_out sum)
  - context: psum_c[1,512] += w_col[lt].T @ feat_nat[lt, :] over 16
    L-tiles (features re-read in natural layout)
"""

import numpy as np
import ml_dtypes

B, L, H, F = 32, 2048, 1024, 1024
NCORES = 8
BLOC = B // NCORES          # 4 batches per core
R = BLOC * L                # 8192 rows per core
RG = 512                    # row-group (matmul moving dim)
NRG = L // RG               # 4 row groups per batch
P = 128
KT = F // P                 # 8 k tiles
MT = H // P                 # 8 m tiles (H output tiles)
LT = L // P                 # 16 L tiles per batch

_COMPILED = {}
LAST_RESULTS = None


def _build():
    import concourse.tile as tile
    from concourse import bacc, mybir

    bf16 = mybir.dt.bfloat16
    f32 = mybir.dt.float32
    AF = mybir.ActivationFunctionType

    nc = bacc.Bacc("TRN2", target_bir_lowering=False, debug=False)

    feat = nc.dram_tensor("feat", [R, F], bf16, kind="ExternalInput").ap()
    wf = nc.dram_tensor("wf", [F, H], bf16, kind="ExternalInput").ap()
    wh = nc.dram_tensor("wh", [H, H], bf16, kind="ExternalInput").ap()
    hidT = nc.dram_tensor("hidT", [H, BLOC], bf16, kind="ExternalInput").ap()
    vin = nc.dram_tensor("vin", [H], bf16, kind="ExternalInput").ap()
    ctx_out = nc.dram_tensor("ctx_out", [BLOC, F], f32, kind="ExternalOutput").ap()
    w_out = nc.dram_tensor("w_out", [BLOC, L], f32, kind="ExternalOutput").ap()

    with tile.TileContext(nc) as tc:
        with (
            tc.tile_pool(name="consts", bufs=1) as consts,
            tc.tile_pool(name="ftT", bufs=3) as ftp,
            tc.tile_pool(name="fnat", bufs=2) as fnp,
            tc.tile_pool(name="tt", bufs=4) as tp,
            tc.tile_pool(name="soft", bufs=2) as sp,
            tc.tile_pool(name="small", bufs=4) as smp,
            tc.tile_pool(name="dram", bufs=2, space="DRAM") as dram,
            tc.tile_pool(name="pf", bufs=2, space="PSUM") as pfp,
            tc.tile_pool(name="pe", bufs=2, space="PSUM") as pep,
            tc.tile_pool(name="pc", bufs=2, space="PSUM") as pcp,
            tc.tile_pool(name="ph", bufs=1, space="PSUM") as php,
        ):
            # --- constants / weights ---
            wf_sb = consts.tile([P, KT, H], bf16)
            nc.sync.dma_start(wf_sb[:], wf.rearrange("(ko p) h -> p ko h", p=P))
            wh_sb = consts.tile([P, KT, H], bf16)
            nc.sync.dma_start(wh_sb[:], wh.rearrange("(ko p) h -> p ko h", p=P))
            hidT_sb = consts.tile([P, KT, BLOC], bf16)
            nc.sync.dma_start(hidT_sb[:], hidT.rearrange("(ko p) b -> p ko b", p=P))
            v_sb = consts.tile([P, MT], bf16)
            nc.sync.dma_start(v_sb[:], vin.rearrange("(ko p) -> p ko", p=P))

            # --- h_projT[H, BLOC] = (hidden @ W_h).T ---
            hprojT = consts.tile([P, MT, BLOC], f32)
            for m in range(MT):
                psum_h = php.tile([P, BLOC], f32)
                for k in range(KT):
                    nc.tensor.matmul(
                        psum_h[:],
                        lhsT=wh_sb[:, k, m * P:(m + 1) * P],
                        rhs=hidT_sb[:, k, :],
                        start=(k == 0),
                        stop=(k == KT - 1),
                    )
                nc.vector.tensor_copy(hprojT[:, m, :], psum_h[:])

            for b in range(BLOC):
                energy = sp.tile([1, L], f32)
                for rg in range(NRG):
                    r0 = b * L + rg * RG
                    ftT = ftp.tile([P, KT, RG], bf16)
                    nc.sync.dma_start_transpose(ftT[:], feat[r0:r0 + RG, :])
                    psum_e = pep.tile([1, RG], f32)
                    for m in range(MT):
                        psum_f = pfp.tile([P, RG], f32)
                        for k in range(KT):
                            nc.tensor.matmul(
                                psum_f[:],
                                lhsT=wf_sb[:, k, m * P:(m + 1) * P],
                                rhs=ftT[:, k, :],
                                start=(k == 0),
                                stop=(k == KT - 1),
                            )
                        t = tp.tile([P, RG], bf16)
                        nc.scalar.activation(
                            t[:], psum_f[:], AF.Tanh, bias=hprojT[:, m, b:b + 1]
                        )
                        nc.tensor.matmul(
                            psum_e[:],
                            lhsT=v_sb[:, m:m + 1],
                            rhs=t[:],
                            start=(m == 0),
                            stop=(m == MT - 1),
                        )
                    nc.vector.tensor_copy(energy[:, rg * RG:(rg + 1) * RG], psum_e[:])

                # --- softmax over L on [1, L] ---
                mx = smp.tile([1, 1], f32)
                nc.vector.tensor_reduce(
                    mx[:], energy[:], axis=mybir.AxisListType.X, op=mybir.AluOpType.max
                )
                nmx = smp.tile([1, 1], f32)
                nc.vector.tensor_scalar_mul(nmx[:], mx[:], -1.0)
                wexp = sp.tile([1, L], f32)
                zsum = smp.tile([1, 1], f32)
                nc.scalar.activation(
                    wexp[:], energy[:], AF.Exp, bias=nmx[:, 0:1], accum_out=zsum[:]
                )
                rz = smp.tile([1, 1], f32)
                nc.vector.reciprocal(rz[:], zsum[:])
                wnorm = sp.tile([1, L], f32)
                nc.vector.tensor_scalar_mul(wnorm[:], wexp[:], rz[:, 0:1])
                nc.sync.dma_start(w_out[b:b + 1, :], wnorm[:])
                wbf = sp.tile([1, L], bf16)
                nc.vector.tensor_scalar_mul(wbf[:], wexp[:], rz[:, 0:1])

                # round-trip through DRAM to flip w to [128, LT] column layout
                wdr = dram.tile([1, L], bf16)
                nc.sync.dma_start(wdr[:], wbf[:])
                wcol = smp.tile([P, LT], bf16)
                nc.sync.dma_start_transpose(
                    wcol[:], wdr.rearrange("o (r c) -> (o r) c", r=LT, c=P)
                )

                # --- context = w @ features[b] ---
                fnat = fnp.tile([P, LT, F], bf16)
                nc.sync.dma_start(
                    fnat[:],
                    feat[b * L:(b + 1) * L, :].rearrange("(lt p) f -> p lt f", p=P),
                )
                ctx_sb = smp.tile([1, F], f32)
                for nf in range(F // RG):
                    psum_c = pcp.tile([1, RG], f32)
                    for lt in range(LT):
                        nc.tensor.matmul(
                            psum_c[:],
                            lhsT=wcol[:, lt:lt + 1],
                            rhs=fnat[:, lt, nf * RG:(nf + 1) * RG],
                            start=(lt == 0),
                            stop=(lt == LT - 1),
                        )
                    nc.vector.tensor_copy(ctx_sb[:, nf * RG:(nf + 1) * RG], psum_c[:])
                nc.sync.dma_start(ctx_out[b:b + 1, :], ctx_sb[:])

    nc.compile()
    return nc


def get_compiled():
    if "nc" not in _COMPILED:
        _COMPILED["nc"] = _build()
    return _COMPILED["nc"]


def kernel(hidden, features, W_h, W_f, V):
    global LAST_RESULTS
    from concourse.bass_utils import run_bass_kernel_spmd

    bf = ml_dtypes.bfloat16
    hidden = np.asarray(hidden, np.float32)
    features = np.asarray(features, np.float32)
    W_h = np.asarray(W_h, np.float32)
    W_f = np.asarray(W_f, np.float32)
    V = np.asarray(V, np.float32)

    feat_b = np.ascontiguousarray(features.astype(bf).reshape(NCORES, R, F))
    wf_b = np.ascontiguousarray(W_f.astype(bf))
    wh_b = np.ascontiguousarray(W_h.astype(bf))
    hidT_b = np.ascontiguousarray(hidden.T.astype(bf))  # [H, B]
    v_b = np.ascontiguousarray(V.astype(bf))

    nc = get_compiled()
    in_maps = [
        {
            "feat": feat_b[i],
            "wf": wf_b,
            "wh": wh_b,
            "hidT": np.ascontiguousarray(hidT_b[:, i * BLOC:(i + 1) * BLOC]),
            "vin": v_b,
        }
        for i in range(NCORES)
    ]
    res = run_bass_kernel_spmd(nc, in_maps, core_ids=list(range(NCORES)))
    LAST_RESULTS = res
    ctx = np.concatenate([res.results[i]["ctx_out"] for i in range(NCORES)], axis=0)
    wts = np.concatenate([res.results[i]["w_out"] for i in range(NCORES)], axis=0)
    return (ctx, wts)
